# revision 8
# baseline (speedup 1.0000x reference)
import os
import sys

sys.path.insert(0, "/opt/trn_rl_repo")
os.environ.setdefault("NEURON_RT_RESET_CORES", "1")

import numpy as np
import ml_dtypes

import concourse.bass as bass
import concourse.bacc as bacc
import concourse.tile as tile
from concourse import mybir

# ---- problem constants (hardcoded; must match reference setup) ----
B, CIN, COUT = 8, 64, 64
E, HEAD, KS = 32, 4, 3
IH = IW = 56
P = IH * IW  # 3136
HP = WP = IH + 2  # padded grid 58x58
PP = HP * WP  # 3364
NCORES = 8
SCALE = float(KS) ** -0.5

F32 = mybir.dt.float32
BF16 = mybir.dt.bfloat16

TPX = 128  # pixels per tile (contiguous span of the padded grid)
NTILES = 26  # ceil(56*58 / 128)
PPA = 60 * WP  # xe_sh padded (extra zero rows) so the last tile's windows stay in-bounds


def _oseg_chunks(j0, j1):
    """Valid-pixel segments of padded pixels [58+j0, 58+j1): (row, col0, len, src_off)."""
    p0, p1 = 58 + j0, min(58 + j1, 58 + 56 * WP)
    segs = []
    r = p0 // WP
    while r * WP < p1 and r <= 56:
        a = max(p0, r * WP + 1)
        b = min(p1, r * WP + 57)
        if b > a and r >= 1:
            segs.append((r, a - r * WP, b - a, a - p0))
        r += 1
    return segs


def _out_segs(t):
    """Valid-pixel segments of tile t: list of (grid_row, col0, len, src_off)."""
    p0 = 58 + TPX * t
    p1 = min(p0 + TPX, 58 + 56 * WP)
    segs = []
    r = p0 // WP
    while r * WP < p1 and r <= 56:
        a = max(p0, r * WP + 1)
        b = min(p1, r * WP + 57)
        if b > a and r >= 1:
            segs.append((r, a - r * WP, b - a, a - p0))
        r += 1
    return segs

NQK = 1056  # q(512) | k(512) | pe(32) columns per dx
NG = 4096  # G columns per dx: (h, c, d)


def _ap(t, dims):
    """View a pool tile with hand-built free-dim [step, count] pairs."""
    return bass.AP(tensor=t.tensor, offset=t.offset, ap=[list(t.ap[0])] + [list(d) for d in dims])


def _apo(t, n, dims):
    """Like _ap but with an extra element offset."""
    return bass.AP(tensor=t.tensor, offset=t.offset + n, ap=[list(t.ap[0])] + [list(d) for d in dims])


def build_program(n_iters=1):
    nc = bacc.Bacc("TRN2", target_bir_lowering=False)

    x_h = nc.dram_tensor("x", [CIN, P], F32, kind="ExternalInput")
    w_in_t_h = nc.dram_tensor("w_in_t", [CIN, E], F32, kind="ExternalInput")
    wqk_h = nc.dram_tensor("wqk", [96, 3 * NQK], BF16, kind="ExternalInput")
    wg_h = nc.dram_tensor("wg", [96, 3 * NG], BF16, kind="ExternalInput")
    w_out_t_h = nc.dram_tensor("w_out_t", [E, COUT], BF16, kind="ExternalInput")
    ident_h = nc.dram_tensor("ident", [128, 128], F32, kind="ExternalInput")
    out_h = nc.dram_tensor("out", [COUT, P], F32, kind="ExternalOutput")

    from contextlib import ExitStack

    with tile.TileContext(nc) as tc:
        with ExitStack() as ctx:
            stage_pool = ctx.enter_context(tc.tile_pool(name="stage", bufs=1))
            const_pool = ctx.enter_context(tc.tile_pool(name="const", bufs=1))
            persist = ctx.enter_context(tc.tile_pool(name="persist", bufs=1))
            qk_pool = ctx.enter_context(tc.tile_pool(name="qk", bufs=2))
            g_pool = ctx.enter_context(tc.tile_pool(name="gsb", bufs=2))
            u1_pool = ctx.enter_context(tc.tile_pool(name="u1p", bufs=1))
            t2_pool = ctx.enter_context(tc.tile_pool(name="t2p", bufs=2))
            l_pool = ctx.enter_context(tc.tile_pool(name="lp", bufs=1))
            e_pool = ctx.enter_context(tc.tile_pool(name="ep", bufs=2))
            tp_pool = ctx.enter_context(tc.tile_pool(name="tpp", bufs=1))
            scr_pool = ctx.enter_context(tc.tile_pool(name="scr", bufs=1))
            small_pool = ctx.enter_context(tc.tile_pool(name="small", bufs=2))
            ps_qkpe_pool = ctx.enter_context(tc.tile_pool(name="ps_qkpe", bufs=1, space="PSUM"))
            ps_g_pool = ctx.enter_context(tc.tile_pool(name="ps_g", bufs=1, space="PSUM"))
            ps_y_pool = ctx.enter_context(tc.tile_pool(name="ps_y", bufs=1, space="PSUM"))
            ps_o_pool = ctx.enter_context(tc.tile_pool(name="ps_o", bufs=1, space="PSUM"))
            ctx.enter_context(nc.allow_low_precision(reason="bf16 attention pipeline"))
            # ---- inputs: x/w_in_t DMA directly (setup-only consumers);
            # loop-read bf16 weights go via stage + compute copy so loop PE
            # instructions never wait directly on multi-queue DMA sems ----
            x_sb = const_pool.tile([CIN, P], F32, tag="x_c")
            nc.sync.dma_start(out=x_sb, in_=x_h[:, :])
            w_in_t = const_pool.tile([CIN, E], F32, tag="w_in_c")
            nc.sync.dma_start(out=w_in_t, in_=w_in_t_h[:, :])

            def launder_bf16(h, parts, cols):
                dstt = const_pool.tile([parts, cols], BF16, tag=h.name + "_c")
                for j0 in range(0, cols, 2048):
                    j1 = min(j0 + 2048, cols)
                    stg = stage_pool.tile([128, 2048], BF16, tag="stg_b")
                    nc.sync.dma_start(out=stg[:parts, :j1 - j0], in_=h[:, j0:j1])
                    nc.vector.tensor_copy(dstt[:, j0:j1], stg[:parts, :j1 - j0])
                return dstt

            wqk = launder_bf16(wqk_h, 96, 3 * NQK)
            wg = launder_bf16(wg_h, 96, 3 * NG)
            w_out_t = launder_bf16(w_out_t_h, E, COUT)
            ident = const_pool.tile([128, 128], F32, tag="ident_c")
            nc.sync.dma_start(out=ident, in_=ident_h[:, :])

            # ---- xe_sh [96, 3364] bf16: partitions (g, c'), where row
            # g*32+c' holds xe[c'] shifted by (g-1) image rows, zero-padded.
            xe_sh = persist.tile([96, PPA], BF16)
            nc.gpsimd.memset(xe_sh, 0.0)
            xe_sh3 = xe_sh.rearrange("p (r w) -> p r w", w=WP)
            for rb in range(7):
                ps_xe = ps_g_pool.tile([E, 448], F32, tag="ps_g")
                nc.tensor.matmul(
                    ps_xe, w_in_t, x_sb[:, rb * 448:(rb + 1) * 448],
                    start=True, stop=True,
                )
                src = ps_xe.rearrange("p (r w) -> p r w", w=IW)
                for g in range(3):
                    r0 = 8 * rb - g + 2
                    eng = nc.scalar.copy if g == 1 else (
                        lambda out, in_: nc.vector.tensor_copy(out, in_))
                    eng(out=xe_sh3[32 * g:32 * g + 32, r0:r0 + 8, 1:57], in_=src)

            out3 = out_h.rearrange("p (r w) -> p r w", w=IW)

            # ---- main loop: 26 128-px tiles, 2-stage software pipeline
            # (frontend of tile t is emitted before backend of tile t-1 so
            # every engine queue always has ready work ahead of it) ----
            def frontend(t):
                f0 = 58 + TPX * t
                ps_qk = ps_qkpe_pool.tile([TPX, 1024], F32, tag="ps_qk")
                for dx in range(3):
                    w0 = dx * NQK
                    st, sp = dx == 0, dx == 2
                    lhsT = xe_sh[:, f0 - 1 + dx: f0 - 1 + dx + TPX]
                    nc.tensor.matmul(ps_qk[:, :512], lhsT, wqk[:, w0:w0 + 512], start=st, stop=sp)
                    nc.tensor.matmul(ps_qk[:, 512:], lhsT, wqk[:, w0 + 512:w0 + 1024], start=st, stop=sp)
                qk_sb = qk_pool.tile([TPX, 1024], BF16, tag="qk")
                nc.scalar.copy(out=qk_sb, in_=ps_qk)
                g_sb = g_pool.tile([TPX, NG], BF16, tag="g")
                for gg in range(2):
                    ps_g = ps_g_pool.tile([TPX, 2048], F32, tag="ps_g")
                    for j in range(4):
                        for dx in range(3):
                            c0 = dx * NG + gg * 2048 + j * 512
                            nc.tensor.matmul(
                                ps_g[:, j * 512:(j + 1) * 512],
                                xe_sh[:, f0 - 1 + dx: f0 - 1 + dx + TPX],
                                wg[:, c0: c0 + 512],
                                start=(dx == 0), stop=(dx == 2),
                            )
                    nc.scalar.copy(out=g_sb[:, gg * 2048:(gg + 1) * 2048], in_=ps_g)
                return (qk_sb, g_sb)

            def backend(t, qk_sb, g_sb, yT_all):
                u1 = u1_pool.tile([TPX, 16384], BF16, tag="u1")
                ll = l_pool.tile([TPX, 4096], BF16, tag="L")
                # te = [ E (h,c,d) | T = E*G ] so one tree reduces S and Z
                te = e_pool.tile([TPX, 8192], BF16, tag="TE")
                ee = _apo(te, 0, [[1, 4096]])
                for hh in range(2):
                    nc.vector.tensor_mul(
                        _apo(u1, 8192 * hh, [[4096, 2], [128, 32], [4, 32], [1, 4]]),
                        _apo(qk_sb, 256 * hh, [[128, 2], [4, 32], [0, 32], [1, 4]]),
                        _apo(qk_sb, 512 + 256 * hh, [[128, 2], [0, 32], [4, 32], [1, 4]]),
                    )
                t2 = t2_pool.tile([TPX, 8192], BF16, tag="t2")
                nc.vector.tensor_add(
                    _ap(t2, [[2, 4096], [1, 2]]),
                    _ap(u1, [[4, 4096], [1, 2]]),
                    _apo(u1, 2, [[4, 4096], [1, 2]]),
                )
                nc.gpsimd.tensor_add(
                    _ap(ll, [[1, 4096]]),
                    _ap(t2, [[2, 4096]]),
                    _apo(t2, 1, [[2, 4096]]),
                )
                nc.scalar.activation(
                    out=ee, in_=_ap(ll, [[1, 4096]]),
                    func=mybir.ActivationFunctionType.Exp, scale=SCALE,
                )
                nc.vector.tensor_mul(
                    _apo(te, 4096, [[1, 4096]]), ee, g_sb)
                # joint S/Z tree over [TPX, 256 groups, 16] halves
                st = scr_pool.tile([TPX, 4096], BF16, tag="st")
                nc.vector.tensor_add(
                    _ap(st, [[16, 256], [1, 16]]),
                    _ap(te, [[32, 256], [1, 16]]),
                    _apo(te, 16, [[32, 256], [1, 16]]),
                )
                for w in (8, 4, 2):
                    nc.vector.tensor_add(
                        _ap(st, [[16, 256], [1, w]]),
                        _ap(st, [[16, 256], [1, w]]),
                        _apo(st, w, [[16, 256], [1, w]]),
                    )
                sz = small_pool.tile([TPX, 256], F32, tag="SZ")
                nc.vector.tensor_add(
                    _ap(sz, [[1, 256]]),
                    _ap(st, [[16, 256]]),
                    _apo(st, 1, [[16, 256]]),
                )
                # y[c] = sum_h S[h,c]/Z[h,c]  (Z = sz[0:128], S = sz[128:256])
                r_sb = small_pool.tile([TPX, 128], F32, tag="R")
                nc.vector.reciprocal_approx_fast(
                    out=r_sb[:, :], in_=_apo(sz, 0, [[1, 128]]))
                yt = small_pool.tile([TPX, 128], F32, tag="ytt")
                nc.vector.tensor_mul(yt, r_sb, _apo(sz, 128, [[1, 128]]))
                y32 = small_pool.tile([TPX, 32], F32, tag="y32")
                nc.vector.tensor_reduce(
                    out=y32,
                    in_=_ap(yt, [[1, 32], [32, 4]]),
                    axis=mybir.AxisListType.X, op=mybir.AluOpType.add,
                )
                ps_yt = ps_y_pool.tile([E, TPX], F32, tag="ps_yt")
                nc.tensor.transpose(ps_yt, y32, ident[:TPX, :TPX])
                nc.scalar.copy(out=yT_all[:, TPX * t: TPX * (t + 1)], in_=ps_yt)

            for _it in range(n_iters):
                yT_all = persist.tile([E, TPX * NTILES], BF16, tag="yT_all")
                pend = None
                for t in range(NTILES):
                    cur = (t, frontend(t))
                    if pend is not None:
                        backend(pend[0], *pend[1], yT_all)
                    pend = cur
                backend(pend[0], *pend[1], yT_all)
                # epilogue: out = w_out^T @ yT for the whole image
                ocols = TPX * NTILES  # 3328 padded-pixel columns
                for j0 in range(0, ocols, 512):
                    j1 = min(j0 + 512, ocols)
                    ps_o = ps_o_pool.tile([COUT, 512], F32, tag="ps_o")
                    nc.tensor.matmul(ps_o[:, :j1 - j0], w_out_t, yT_all[:, j0:j1],
                                     start=True, stop=True)
                    o_sb = small_pool.tile([COUT, 512], F32, tag="o_sb")
                    nc.scalar.copy(out=o_sb[:, :j1 - j0], in_=ps_o[:, :j1 - j0])
                    for (r, c0, ln, so) in _oseg_chunks(j0, j1):
                        nc.sync.dma_start(
                            out=out3[:, r - 1, c0 - 1: c0 - 1 + ln],
                            in_=o_sb[:, so: so + ln],
                        )

    if not nc.is_finalized():
        nc.finalize()
    return nc


def _bf16(a):
    return np.asarray(a, np.float32).astype(ml_dtypes.bfloat16)


def _prep_weights(w_in, w_q, w_k, w_v, w_pe, w_p1, w_out):
    w_q = np.asarray(w_q, np.float32)
    w_k = np.asarray(w_k, np.float32)
    w_v = np.asarray(w_v, np.float32)
    w_pe = np.asarray(w_pe, np.float32)
    w_p1 = np.asarray(w_p1, np.float32)

    # wqk[(dy,c'), (dx, [q|k|pe])]
    wqk = np.zeros((3, 3, 32, NQK), np.float32)  # [dy, dx, c', col]
    for h in range(HEAD):
        for k in range(KS):
            for c in range(E):
                oc = c * (HEAD * KS) + h * KS + k
                # q block: col (h, c, k4); contraction row c'=c
                wqk[:, :, c, h * 128 + c * 4 + k] = w_q[oc, 0, :, :]
                # k block: col (h, d, k4); source channel d=c
                wqk[:, :, c, 512 + h * 128 + c * 4 + k] = w_k[oc, 0, :, :]
    # -> [96=(dy,c'), 3*NQK=(dx, col)]
    wqk = wqk.transpose(0, 2, 1, 3).reshape(96, 3 * NQK)

    # WG[(dy,d'), (dx, (h,c,d))]: W2[c,h,d,dy,dx] = sum_k p1[c,h*3+k]*wv[d*12+h*3+k,0,dy,dx]
    wgm = np.zeros((3, 3, 32, NG), np.float32)  # [dy, dx, d', col]
    for h in range(HEAD):
        for d in range(E):
            vv = w_v[d * (HEAD * KS) + h * KS: d * (HEAD * KS) + h * KS + KS, 0, :, :]
            for c in range(E):
                w2 = np.einsum("k,kyx->yx", w_p1[c, h * KS:(h + 1) * KS], vv)
                wgm[:, :, d, h * 1024 + c * 32 + d] = w2
    # fold the positional-encoding residual: sum_d attn[h,c,d] == 1 per
    # (h,c), so adding pe_conv[c]/HEAD to every (h,c,d) slot of G adds
    # exactly pe[c] to y. pe_conv[c] contracts row (dy, d'=c).
    for h in range(HEAD):
        for c in range(E):
            for d in range(E):
                wgm[:, :, c, h * 1024 + c * 32 + d] += w_pe[c, 0, :, :] / HEAD
    wg = wgm.transpose(0, 2, 1, 3).reshape(96, 3 * NG)

    return {
        "w_in_t": np.ascontiguousarray(np.asarray(w_in, np.float32).T),
        "wqk": _bf16(wqk),
        "wg": _bf16(wg),
        "w_out_t": _bf16(np.asarray(w_out, np.float32).T),
        "ident": np.eye(128, dtype=np.float32),
    }


_NC_CACHE = {}


def kernel(x, w_in, w_q, w_k, w_v, w_pe, w_p1, w_out):
    from concourse.bass_utils import run_bass_kernel_spmd

    x = np.asarray(x, np.float32)
    weights = _prep_weights(w_in, w_q, w_k, w_v, w_pe, w_p1, w_out)
    if "nc" not in _NC_CACHE:
        _NC_CACHE["nc"] = build_program()
    nc = _NC_CACHE["nc"]

    in_maps = []
    for i in range(NCORES):
        m = dict(weights)
        m["x"] = np.ascontiguousarray(x[i].reshape(CIN, P))
        in_maps.append(m)

    res = run_bass_kernel_spmd(nc, in_maps, list(range(NCORES)))
    outs = [res.results[i]["out"].reshape(COUT, IH, IW) for i in range(NCORES)]
    return np.stack(outs, axis=0)


if __name__ == "__main__":
    nc = build_program()
    print("program built ok")
